# revision 11
# baseline (speedup 1.0000x reference)
"""Causal dilated 1D conv (B=16, C=32, L=131072, KW=3, dil=4, left-pad 8)
as a Bass/Tile kernel on 8 Trainium2 NeuronCores.

Strategy
--------
Data-parallel: batch dim 16 -> 2 batches per core; weights replicated.

Per core the conv is computed as 3 shifted matmuls accumulated in PSUM:
    out[:, t] = sum_k  W_k @ x[:, t - 8 + 4k],   W_k in R^{32x32}.

With only 32 channels a plain matmul would use 32 of 128 partitions.  We
instead pack 4 consecutive column chunks of the sequence into 4 partition
groups (32 channels each) and make the stationary operand a 128x128
block-diagonal replication of W_k^T (built on the host), so one matmul
processes 4 chunks at once; the three tap matmuls accumulate in PSUM.

Partition p holds (channel c = p//4, group g = p%4) — channel-major so the
outermost DMA access-pattern dim is 32 wide, which the hardware DGE splits
across all 16 SDMA engines (group-major only got 4 engines = ~100 GB/s).

x is left-padded with PAD zero columns on the host, so every tile's halo
load is uniform (no edge case).  Matmul inputs use dtype float32r (same
fp32 bits; full-rate PE instead of fp32's 1/4 rate).  fp32 PSUM accumulate.
"""

import numpy as np

import concourse.bass as bass
import concourse.mybir as mybir
from concourse.tile import TileContext
from concourse.bass_utils import run_bass_kernel_spmd

# Problem constants (hardcoded per harness contract).
B, C, L = 16, 32, 131072
KW, DIL, PAD = 3, 4, 8

N_CORES = 8
B_PER_CORE = B // N_CORES          # 2
GROUPS = 4                         # partition groups of 32 channels
GTILE = 4096                       # columns per group per tile
SPAN = GROUPS * GTILE              # 8192 columns consumed per tile
HALO = PAD                         # left halo = (KW-1)*DIL
LP = L + PAD                       # padded row length in DRAM
NTILES = L // SPAN                 # 16 tiles per batch
MM_N = 512                         # matmul moving dim (one PSUM bank, fp32)
NSUB = GTILE // MM_N               # psum tiles per group-chunk
PSUM_CHUNK = 8                     # psum banks usable per accumulate round

DT_MM = mybir.dt.float16           # matmul input dtype (fp32 PSUM accumulate)
DT_OUT = mybir.dt.float16          # device-side output dtype (host upcasts)
NP_MM = np.float16


def _split_sync_waits(nc: bass.Bass, max_waits: int = 1) -> None:
    """The walrus build in this container rejects >`max_waits` sync-waits on
    an instruction.  Hoist excess waits onto fresh NoOp instructions inserted
    just before the offender on the same engine — program order on one engine
    serializes them, so semantics are unchanged."""
    ctr = 0
    for f in nc.m.functions:
        for bb in f.blocks:
            insts = bb.instructions
            new = []
            for inst in insts:
                si = getattr(inst, "sync_info", None)
                if si is not None and si.on_wait and len(si.on_wait) > max_waits:
                    waits = list(si.on_wait)
                    head, keep = waits[:-max_waits], waits[-max_waits:]
                    for w in head:
                        nop = mybir.InstNoOp(
                            name=f"splitw-{ctr}",
                            engine=inst.engine,
                            bass_nofuse=True,
                            sync_info=mybir.SyncInfo(on_wait=[w], on_update=[]),
                        )
                        ctr += 1
                        new.append(nop)
                    inst.sync_info = mybir.SyncInfo(
                        on_wait=keep, on_update=list(si.on_update or [])
                    )
                new.append(inst)
            insts[:] = new


def _build_nc() -> bass.Bass:
    nc = bass.Bass(target_bir_lowering=False, trn_type="TRN2")
    ROWS = B_PER_CORE * C  # 64 DRAM rows per core
    x = nc.dram_tensor("x", [ROWS, LP], DT_MM, kind="ExternalInput")
    w = nc.dram_tensor("w", [128, KW, 128], DT_MM, kind="ExternalInput")
    out = nc.dram_tensor("out", [ROWS, L], DT_OUT, kind="ExternalOutput")

    with TileContext(nc) as tc:
        with (
            tc.tile_pool(name="wpool", bufs=1) as wpool,
            tc.tile_pool(name="xpool", bufs=4) as xpool,
            tc.tile_pool(name="opool", bufs=3) as opool,
            tc.tile_pool(name="psum", bufs=1, space="PSUM") as psum,
        ):
            wt = wpool.tile([128, KW, 128], DT_MM)
            nc.sync.dma_start(out=wt[:], in_=w[:])

            for b in range(B_PER_CORE):
                base = b * C * LP       # element offset into padded x slab
                obase = b * C * L       # element offset into out slab
                for i in range(NTILES):
                    t0 = i * SPAN
                    xt = xpool.tile([128, HALO + GTILE], DT_MM)
                    # partition p = c*4 + g reads x_pad[c, t0 + g*GTILE + v],
                    # v in [0, HALO+GTILE) — t0 is already halo-shifted by the
                    # host-side PAD zeros at the row start.
                    nc.sync.dma_start(
                        out=xt[:],
                        in_=bass.AP(
                            x,
                            base + t0,
                            [[LP, C], [GTILE, GROUPS], [1, HALO + GTILE]],
                        ),
                    )

                    ot = opool.tile([128, GTILE], DT_OUT)
                    for jc in range(0, NSUB, PSUM_CHUNK):
                        jn = min(PSUM_CHUNK, NSUB - jc)
                        pts = [
                            psum.tile([128, MM_N], mybir.dt.float32, name=f"pt{jj}")
                            for jj in range(jn)
                        ]
                        for k in range(KW):
                            lhsT = wt[:, k, :]
                            for jj in range(jn):
                                off = (jc + jj) * MM_N + k * DIL
                                nc.tensor.matmul(
                                    out=pts[jj][:],
                                    lhsT=lhsT,
                                    rhs=xt[:, off : off + MM_N],
                                    start=(k == 0),
                                    stop=(k == KW - 1),
                                )
                        for jj in range(jn):
                            j = jc + jj
                            dst = ot[:, j * MM_N : (j + 1) * MM_N]
                            if jj % 4 == 3:
                                nc.scalar.copy(out=dst, in_=pts[jj][:])
                            else:
                                nc.vector.tensor_copy(out=dst, in_=pts[jj][:])

                    nc.scalar.dma_start(
                        out=bass.AP(
                            out, obase + t0, [[L, C], [GTILE, GROUPS], [1, GTILE]]
                        ),
                        in_=ot[:],
                    )
    _split_sync_waits(nc)
    return nc


_NC_CACHE = None


def _get_nc() -> bass.Bass:
    global _NC_CACHE
    if _NC_CACHE is None:
        _NC_CACHE = _build_nc()
    return _NC_CACHE


def kernel(x: np.ndarray, W: np.ndarray, _trace: bool = False):
    x = np.ascontiguousarray(x, dtype=np.float32)   # (16, 32, 131072)
    W = np.ascontiguousarray(W, dtype=np.float32)   # (32, 96)

    # Left-pad with the causal zeros so the device sees a uniform halo.
    x_pad = np.zeros((B, C, LP), dtype=NP_MM)
    x_pad[:, :, PAD:] = x.astype(NP_MM)

    # Stationary weights for partition layout p = c*4 + g:
    #   w_bd[ci*4+g, k, co*4+g] = W[co, 3*ci+k]
    Wk = W.reshape(C, C, KW)                        # (co, ci, k)
    blk = Wk.transpose(1, 2, 0)                     # (ci, k, co)
    w_bd = np.zeros((128, KW, 128), dtype=NP_MM)
    view = w_bd.reshape(C, GROUPS, KW, C, GROUPS)
    for g in range(GROUPS):
        view[:, g, :, :, g] = blk

    nc = _get_nc()
    in_maps = []
    for core in range(N_CORES):
        xs = x_pad[core * B_PER_CORE : (core + 1) * B_PER_CORE].reshape(
            B_PER_CORE * C, LP
        )
        in_maps.append({"x": np.ascontiguousarray(xs), "w": w_bd})

    res = run_bass_kernel_spmd(
        nc, in_maps, core_ids=list(range(N_CORES)), trace=_trace
    )

    out = np.empty((B, C, L), dtype=np.float32)
    for core in range(N_CORES):
        out[core * B_PER_CORE : (core + 1) * B_PER_CORE] = res.results[core][
            "out"
        ].reshape(B_PER_CORE, C, L).astype(np.float32)
    if _trace:
        return out, res
    return out
